# revision 70
# baseline (speedup 1.0000x reference)
# Multi-head attention (B=4, S=2048, D=1024, H=16) on 8 NeuronCores.
#
# Sharding: batch x head-group. Core c handles batch b=c//2 and heads
# 8*(c%2) .. 8*(c%2)+7 (a 512-wide slice of the model dim). Each core
# computes QKV projections for its slice, causal attention for its 8
# heads, and a row-parallel partial of the output projection. The host
# sums the two partials per batch and adds bo.
#
# Arithmetic / engine layout (v2, fp8 + flipped AV):
# - Q/K projections run as fp8e4m3 DoubleRow matmuls (2 contraction
#   k-tiles per instruction, 0.5 PE cycles/col): x8 = e4m3(x) and
#   W8 = e4m3(32 W) are quantized on the host. The bias add (DVE)
#   rescales PSUM by 1/2 and adds 16 b, storing 16(q+b) as fp8e4m3.
# - Scores also run as fp8 DoubleRow: the 64-dim head contraction is
#   split as 32 partitions x 2 rows. The W columns of Wq/Wk are permuted
#   host-side so each head's dims land as [32 partitions, 2 rows]. The
#   score PSUM is 256x the true scores; exp folds SCALE/256 into the
#   activation scale.
# - The AV matmul is flipped vs the baseline: out[q-part, c-free] with
#   et (bf16 probs) stationary and v (bf16, 64 cols + 1 ones-col for the
#   softmax denominator) moving: cost is 65 cols/block instead of 512.
#   The denominator lands per-partition, so normalization is a [128,4]
#   DVE reciprocal + per-head DVE tensor_scalar_mul - no ACT work.
# - Normalized ctx [q, c] goes back to [c, q] via bf16 SBUF->SBUF DMA
#   crossbar transposes (no engine time) to feed the row-parallel output
#   projection (bf16, unchanged from baseline). V projection runs bf16.
# - Engine legality: GPSIMD/Pool may not touch PSUM, so every
#   PSUM-reading op (biases, norm, output copies) sits on DVE; Pool gets
#   the SBUF-only causal tri-mask multiplies.
# - exp stays on ACT - the bottleneck engine (~152us busy: 147k
#   elem/partition + a ~190ns access bubble per instruction). Emission
#   interleaves DMA issue order with compute (DMA semaphores are
#   queue-cumulative), lags AV two heads behind scores, and spreads
#   projections/output-projection chunks into PE stalls between heads.
#
# Causality: fully-masked k-blocks are skipped, the exp of diagonal
# groups is trimmed to the valid column range, and the in-block triangle
# is zeroed with one [128,128] upper-tri mask multiply per diag block.
# Timeline sim: 214.3us vs 275.0us baseline (ACT-bound; PE ~133us,
# DVE ~90us, Pool ~50us busy). V projections run as just-in-time
# carries at the head of the phase whose AV consumes them, so each
# phase boundary leaves PE nearly empty for the next scores; fill
# slots are capped at one output-projection or projection piece each,
# because the exp read-ahead is limited to 2 score groups (~2.1us) by
# PSUM, and any longer PE burst between heads idles ACT.

import sys

for _p in ("/opt/trn_rl_repo", "/root/.axon_site/_ro/trn_rl_repo"):
    if _p not in sys.path:
        sys.path.append(_p)

import ml_dtypes
import numpy as np

import concourse.bass as bass
import concourse.mybir as mybir
import concourse.tile as tile
from concourse.bass_utils import run_bass_kernel_spmd
from concourse.masks import make_upper_triangular, make_identity

B, S, D, H = 4, 2048, 1024, 16
HD = D // H            # 64
N_CORES = 8
GH = 8                 # heads per core
C = GH * HD            # 512 local model dims per core
SCALE = HD ** -0.5
F32 = mybir.dt.float32
F32R = mybir.dt.float32r
BF16 = mybir.dt.bfloat16
E4 = mybir.dt.float8e4

T_CHUNK = 256          # t-tile for QKV projections
N_TC = 2048 // T_CHUNK # 8 t-chunks
QC = 512               # q columns per attention chunk
KB = 128               # k rows per attention block
N_KB = S // KB         # 16
N_QC = S // QC         # 4
EXP_GROUP = 2          # k-blocks per batched exp (2 psum banks x 2 bufs)

NFC = D // 128         # 8 f-chunks of the projection contraction
NFP = NFC // 2         # 4 DoubleRow f-pairs
NCC = C // 128         # 4 c-chunks of the local model dim

# fp8 scaling: W8 = e4m3(32 W), x8 = e4m3(x)  =>  psum = 32 q_nb
# stored q̂ = 16(q+b) = psum * 0.5 + 16 b ; score psum = 256 s
W_SCALE = 32.0
QK_STORE = 16.0
PS_TO_STORE = QK_STORE / W_SCALE            # 0.5
EXP_SCALE = SCALE / (QK_STORE * QK_STORE)   # fold 1/256 into exp


def _split_multi_waits(nc):
    """walrus in this container accepts only one sync-wait per instruction.
    Hoist all but the last wait of any multi-wait instruction onto NoOps
    inserted just before it on the same engine (sequencers execute their
    queue in order, so chained single waits are equivalent)."""
    for f in nc.m.functions:
        for blk in f.blocks:
            new_insts = []
            for inst in blk.instructions:
                si = inst.sync_info
                if si is not None and si.on_wait and len(si.on_wait) > 1:
                    waits = list(si.on_wait)
                    for i, w in enumerate(waits[:-1]):
                        nop = mybir.InstNoOp(name=f"{inst.name}_sw{i}", ins=[], outs=[])
                        nop.engine = inst.engine
                        nop.sync_info = mybir.SyncInfo(on_wait=[w], on_update=[])
                        new_insts.append(nop)
                    si.on_wait = [waits[-1]]
                new_insts.append(inst)
            blk.instructions[:] = new_insts


def _emit_kernel(nc, reps=1):
    xbt = nc.dram_tensor("xbt", [D, S], BF16, kind="ExternalInput").ap()
    x8t = nc.dram_tensor("x8t", [D, S], E4, kind="ExternalInput").ap()
    wq8 = nc.dram_tensor("wq8", [D, C], E4, kind="ExternalInput").ap()
    wk8 = nc.dram_tensor("wk8", [D, C], E4, kind="ExternalInput").ap()
    wvt = nc.dram_tensor("wvt", [D, C], BF16, kind="ExternalInput").ap()
    bqv = nc.dram_tensor("bqv", [128, NCC], F32, kind="ExternalInput").ap()
    bkv = nc.dram_tensor("bkv", [128, NCC], F32, kind="ExternalInput").ap()
    bvb = nc.dram_tensor("bvb", [128, C], F32, kind="ExternalInput").ap()
    wot = nc.dram_tensor("wot", [C, D], BF16, kind="ExternalInput").ap()
    out = nc.dram_tensor("out", [S, D], F32, kind="ExternalOutput").ap()

    with tile.TileContext(nc) as tc:
        import contextlib

        ctx = contextlib.ExitStack()
        with ctx:
            consts = ctx.enter_context(tc.tile_pool(name="consts", bufs=1))
            wpool = ctx.enter_context(tc.tile_pool(name="wpool", bufs=1))
            qkv = ctx.enter_context(tc.tile_pool(name="qkv", bufs=1))
            xtp = ctx.enter_context(tc.tile_pool(name="xtp", bufs=4))
            x8p = ctx.enter_context(tc.tile_pool(name="x8p", bufs=4))
            etp = ctx.enter_context(tc.tile_pool(name="etp", bufs=3))
            ctxnp = ctx.enter_context(tc.tile_pool(name="ctxnp", bufs=2))
            ctxp = ctx.enter_context(tc.tile_pool(name="ctxp", bufs=1))
            smallp = ctx.enter_context(tc.tile_pool(name="smallp", bufs=4))
            outp = ctx.enter_context(tc.tile_pool(name="outp", bufs=3))

            ps_qkv = ctx.enter_context(
                tc.tile_pool(name="ps_qkv", bufs=2, space="PSUM")
            )
            ps_sc = ctx.enter_context(
                tc.tile_pool(name="ps_sc", bufs=2, space="PSUM")
            )
            ps_av = ctx.enter_context(
                tc.tile_pool(name="ps_av", bufs=2, space="PSUM")
            )

            # ---- constants (tiles only; mask gen is emitted after the
            # prologue weight DMAs so it never heads the Pool queue) ----------
            tri = consts.tile([128, 128], BF16)      # tri[p, c] = 1.0 iff p <= c

            bv_bc = consts.tile([128, GH, HD], F32)  # bv broadcast across partitions

            bq_sb = consts.tile([128, NCC], F32)     # 16*bq[perm] at [p, cc]
            bk_sb = consts.tile([128, NCC], F32)

            # ---- weights + early input chunks ------------------------------
            # DMA semaphores are queue-cumulative (a consumer waits for
            # everything issued earlier on its queue), so DMAs are issued
            # interleaved with the compute that consumes them, in strict
            # first-need order per queue.
            wq_sb = wpool.tile([128, NFC, C], E4)
            wk_sb = wpool.tile([128, NFC, C], E4)
            wv_sb = wpool.tile([128, NFC, C], BF16)
            wo_sb = wpool.tile([128, NCC, D], BF16)

            _pref = {}

            def issue_x8_dma(tci):
                t0 = tci * T_CHUNK
                x8_c = x8p.tile([128, NFC, T_CHUNK], E4, name="x8_c")
                nc.sync.dma_start(
                    out=x8_c[:, :, :],
                    in_=x8t.rearrange("(fc p) t -> p fc t", p=128)[:, :, t0 : t0 + T_CHUNK],
                )
                _pref[tci] = (None, x8_c)

            def issue_xb_dma(tci):
                t0 = tci * T_CHUNK
                xb_c = xtp.tile([128, NFC, T_CHUNK], BF16, name="xb_c")
                nc.sync.dma_start(
                    out=xb_c[:, :, :],
                    in_=xbt.rearrange("(fc p) t -> p fc t", p=128)[:, :, t0 : t0 + T_CHUNK],
                )
                _pref[tci] = (xb_c, _pref[tci][1])

            def issue_chunk_dmas(tci):
                issue_x8_dma(tci)
                issue_xb_dma(tci)

            # ---- persistent activations -----------------------------------
            qt_sb = qkv.tile([128, NCC, S], E4)      # q̂: [32p x 2row per head]
            kt_sb = qkv.tile([128, NCC, S], E4)
            v_sb = qkv.tile([128, N_KB, GH, HD + 1], BF16)  # v + ones col
            ctx_sb = ctxp.tile([128, NCC, S], BF16)  # ctxT: [c within chunk, cc, q]

            def emit_qk_proj(tci, which, bias_engine="vector"):
                t0 = tci * T_CHUNK
                x8_c = _pref[tci][1]
                w_sb, b_sb, y_sb = (
                    (wq_sb, bq_sb, qt_sb) if which == "q" else (wk_sb, bk_sb, kt_sb)
                )
                for cc in range(NCC):
                    ps = ps_qkv.tile(
                        [128, T_CHUNK], F32, name=f"ps_{which}", tag="ps_qkv"
                    )
                    for i in range(NFP):
                        nc.tensor.matmul(
                            ps[:, :],
                            w_sb[:, 2 * i : 2 * i + 2, cc * 128 : (cc + 1) * 128],
                            x8_c[:, 2 * i : 2 * i + 2, :],
                            start=(i == 0),
                            stop=(i == NFP - 1),
                            perf_mode=mybir.MatmulPerfMode.DoubleRow,
                        )
                    if bias_engine == "scalar":
                        # prologue only: ACT is idle before the first exp,
                        # and the serial bias chain is the critical path
                        # to the first scores
                        nc.scalar.activation(
                            y_sb[:, cc, t0 : t0 + T_CHUNK],
                            ps[:, :],
                            mybir.ActivationFunctionType.Identity,
                            bias=b_sb[:, cc : cc + 1],
                            scale=PS_TO_STORE,
                        )
                    else:
                        eng = nc.vector if bias_engine == "vector" else nc.gpsimd
                        eng.tensor_scalar(
                            out=y_sb[:, cc, t0 : t0 + T_CHUNK],
                            in0=ps[:, :],
                            scalar1=PS_TO_STORE,
                            scalar2=b_sb[:, cc : cc + 1],
                            op0=mybir.AluOpType.mult,
                            op1=mybir.AluOpType.add,
                        )

            def emit_v_proj(tci, tt):
                t0 = tci * T_CHUNK
                xb_c = _pref[tci][0]
                kb = (t0 + tt * 128) // KB
                ps = ps_qkv.tile([128, C], F32, name="ps_v", tag="ps_qkv")
                for fc in range(NFC):
                    nc.tensor.matmul(
                        ps[:, :],
                        xb_c[:, fc, tt * 128 : (tt + 1) * 128],
                        wv_sb[:, fc, :],
                        start=(fc == 0),
                        stop=(fc == NFC - 1),
                    )
                nc.vector.tensor_add(
                    v_sb[:, kb, :, 0:HD],
                    ps.rearrange("p (h d) -> p h d", h=GH),
                    bv_bc[:, :, :],
                )

            def emit_scores(h, qi):
                hp = 32 * (h % 4)        # partition offset of this head
                hc = 2 * (h // 4)        # first of the head's 2 cc rows
                q0 = qi * QC
                nkb = 4 * qi + 4         # causal: k-blocks 0 .. 4qi+3
                et = etp.tile([128, N_KB, QC], BF16, name="et")
                n_grp = (nkb + EXP_GROUP - 1) // EXP_GROUP
                for gi in range(n_grp):
                    kb_lo = gi * EXP_GROUP
                    kb_hi = min(kb_lo + EXP_GROUP, nkb)
                    gw = kb_hi - kb_lo
                    sc_ps = ps_sc.tile([128, EXP_GROUP, QC], F32)
                    for kb in range(kb_lo, kb_hi):
                        nc.tensor.matmul(
                            sc_ps[:, kb - kb_lo, :],
                            kt_sb[hp : hp + 32, hc : hc + 2, kb * KB : (kb + 1) * KB],
                            qt_sb[hp : hp + 32, hc : hc + 2, q0 : q0 + QC],
                            start=True,
                            stop=True,
                            perf_mode=mybir.MatmulPerfMode.DoubleRow,
                            tile_position=(hp, 0),
                        )
                    # cols < 128*m of diagonal block m are never read by
                    # AV; a rectangular trim to the group's min offset is
                    # safe and cuts ACT work on the causal tail.
                    g_min_m = kb_lo - 4 * qi
                    g_off = 128 * g_min_m if g_min_m > 0 else 0
                    nc.scalar.activation(
                        et[:, kb_lo:kb_hi, g_off:QC],
                        sc_ps[:, 0:gw, g_off:QC],
                        mybir.ActivationFunctionType.Exp,
                        bias=0.0,
                        scale=EXP_SCALE,
                    )
                    for kb in range(kb_lo, kb_hi):
                        m = kb - 4 * qi  # >= 0 on the causal diagonal
                        if m >= 0:
                            # SBUF-only op: Pool can take it (it may not
                            # touch PSUM), keeping DVE for the PSUM readers
                            nc.gpsimd.tensor_mul(
                                et[:, kb, 128 * m : 128 * m + 128],
                                et[:, kb, 128 * m : 128 * m + 128],
                                tri[:, :],
                            )
                return et

            def emit_av_norm(h, qi, et, ctxn):
                # flipped AV: out[q-part, c-free], v moving (64 + ones col)
                av_ps = ps_av.tile([128, 4, HD + 1], F32, name="av", tag="ps_av")
                for mq in range(4):
                    qb = 4 * qi + mq
                    for kb in range(qb + 1):
                        nc.tensor.matmul(
                            av_ps[:, mq, :],
                            et[:, kb, 128 * mq : 128 * mq + 128],
                            v_sb[:, kb, h, :],
                            start=(kb == 0),
                            stop=(kb == qb),
                        )
                den = smallp.tile([128, 4], F32, name="den")
                nc.vector.tensor_copy(den[:, :], av_ps[:, :, HD])
                rec = smallp.tile([128, 4], F32, name="rec")
                nc.vector.reciprocal(rec[:, :], den[:, :])
                for mq in range(4):
                    nc.vector.tensor_scalar_mul(
                        ctxn[:, mq, h, :],
                        av_ps[:, mq, 0:HD],
                        rec[:, mq : mq + 1],
                    )

            def emit_transposes(qi, ctxn, mqs=range(4), hpairs=range(NCC)):
                # bf16 SBUF->SBUF transpose on the DMA crossbar: no engine
                # time at all (the sync queue carries the descriptors)
                for mq in mqs:
                    q0 = (4 * qi + mq) * 128
                    for hpair in hpairs:
                        nc.sync.dma_start_transpose(
                            ctx_sb[:, hpair, q0 : q0 + 128],
                            ctxn[:, mq, 2 * hpair : 2 * hpair + 2, :],
                        )

            def emit_phase3(qq, tail=False):
                for eh in range(2):
                    ps = ps_qkv.tile([128, D // 2], F32, name="ps_op", tag="ps_qkv")
                    for cc in range(NCC):
                        nc.tensor.matmul(
                            ps[:, :],
                            ctx_sb[:, cc, qq * 128 : (qq + 1) * 128],
                            wo_sb[:, cc, eh * (D // 2) : (eh + 1) * (D // 2)],
                            start=(cc == 0),
                            stop=(cc == NCC - 1),
                        )
                    o_sb = outp.tile([128, D // 2], F32, name="o_sb")
                    nc.vector.tensor_copy(o_sb[:, :], ps[:, :])
                    # at the drain tail spread the final stores across two
                    # queues (ACT is idle then); mid-stream keep them off
                    # the scalar queue so they never gate a chunk DMA
                    dma_q = nc.scalar if (tail and eh == 1) else nc.sync
                    dma_q.dma_start(
                        out=out[qq * 128 : (qq + 1) * 128, eh * (D // 2) : (eh + 1) * (D // 2)],
                        in_=o_sb[:, :],
                    )

            def emit_wo_dmas():
                for cc in range(NCC):
                    nc.scalar.dma_start(
                        out=wo_sb[:, cc, :], in_=wot[cc * 128 : (cc + 1) * 128, :]
                    )

            AV_LAG = 2  # AV trails scores by 2 heads (et pool bufs = LAG+1)

            def emit_attention(qi, prev_ctxn):
                """Heads of q-chunk qi with PE filler work interleaved.
                Scores(h) go first each slot (they feed ACT, the
                bottleneck); AV+norm lag AV_LAG heads so the V
                projections each AV needs are already queued; the
                transposes + output projection of qi-1 and the
                projections for qi+1 fill PE stalls between heads."""
                # V projections for THIS qi's new k-blocks run as a
                # just-in-time carry at h1/h2 (before AV(h0) at the lag
                # slot), so the end of each phase leaves PE nearly empty
                # and the next phase's first scores issue immediately.
                t_a, t_b = 2 * qi + 2, 2 * qi + 3  # next qi's chunks
                op0 = 4 * (qi - 1)
                if qi == 0:
                    fill = [
                        [lambda: emit_v_proj(0, 0), lambda: emit_v_proj(0, 1)],
                        [lambda: emit_v_proj(1, 0), lambda: emit_v_proj(1, 1)],
                        [lambda: issue_chunk_dmas(t_a),
                         lambda: issue_chunk_dmas(t_b),
                         emit_wo_dmas],
                        [lambda: emit_qk_proj(t_a, "q")],
                        [lambda: emit_qk_proj(t_a, "k")],
                        [lambda: emit_qk_proj(t_b, "q")],
                        [lambda: emit_qk_proj(t_b, "k")],
                        [lambda: issue_chunk_dmas(t_a + 2),
                         lambda: issue_chunk_dmas(t_b + 2)],
                    ]
                elif qi < N_QC - 1:
                    more = 2 * qi + 4 < N_TC
                    fill = [
                        [lambda: emit_transposes(qi - 1, prev_ctxn, (0, 1)),
                         lambda: emit_v_proj(2 * qi, 0),
                         lambda: emit_v_proj(2 * qi, 1)],
                        [lambda: emit_transposes(qi - 1, prev_ctxn, (2, 3)),
                         lambda: emit_v_proj(2 * qi + 1, 0),
                         lambda: emit_v_proj(2 * qi + 1, 1)],
                        [lambda: emit_phase3(op0)],
                        [lambda: emit_phase3(op0 + 1), lambda: emit_qk_proj(t_a, "q")],
                        [lambda: emit_phase3(op0 + 2), lambda: emit_qk_proj(t_a, "k")],
                        [lambda: emit_phase3(op0 + 3), lambda: emit_qk_proj(t_b, "q")],
                        [lambda: emit_qk_proj(t_b, "k")],
                        ([lambda: issue_chunk_dmas(2 * qi + 4),
                          lambda: issue_chunk_dmas(2 * qi + 5)] if more else []),
                    ]
                else:
                    fill = [
                        [lambda: emit_transposes(qi - 1, prev_ctxn, (0, 1)),
                         lambda: emit_v_proj(2 * qi, 0),
                         lambda: emit_v_proj(2 * qi, 1)],
                        [lambda: emit_transposes(qi - 1, prev_ctxn, (2, 3)),
                         lambda: emit_v_proj(2 * qi + 1, 0),
                         lambda: emit_v_proj(2 * qi + 1, 1)],
                        [lambda: emit_phase3(op0)],
                        [lambda: emit_phase3(op0 + 1)],
                        [lambda: emit_phase3(op0 + 2)],
                        [lambda: emit_phase3(op0 + 3)],
                    ]

                # transposes chase completed head pairs on the last qi
                last = qi == N_QC - 1
                lag = AV_LAG
                ctxn = ctxnp.tile([128, 4, GH, HD], BF16, name="ctxn", tag="ctxn")

                def after_norm(hn):
                    if last and hn % 2 == 1:
                        emit_transposes(qi, ctxn, hpairs=(hn // 2,))

                ets = {}
                for h in range(GH):
                    ets[h] = emit_scores(h, qi)
                    if h >= 1 and h - 1 < len(fill):
                        for f in fill[h - 1]:
                            f()
                    if h >= lag:
                        emit_av_norm(h - lag, qi, ets.pop(h - lag), ctxn)
                        after_norm(h - lag)
                for slot in fill[GH - 1 :]:
                    for f in slot:
                        f()
                for h in range(GH - lag, GH):
                    emit_av_norm(h, qi, ets.pop(h), ctxn)
                    after_norm(h)
                return ctxn

            for _rep in range(reps):
                # critical path first: x8(t0,t1), wq8, bq -> q projections;
                # then the k side; mask gen and the V/O weight queue follow.
                issue_x8_dma(0)
                issue_x8_dma(1)
                nc.gpsimd.dma_start(
                    out=wq_sb[:, :, :], in_=wq8.rearrange("(fc p) c -> p fc c", p=128)
                )
                nc.scalar.dma_start(out=bq_sb[:, :], in_=bqv)
                emit_qk_proj(0, "q")
                emit_qk_proj(1, "q")
                nc.gpsimd.dma_start(
                    out=wk_sb[:, :, :], in_=wk8.rearrange("(fc p) c -> p fc c", p=128)
                )
                nc.scalar.dma_start(out=bk_sb[:, :], in_=bkv)
                emit_qk_proj(0, "k")
                emit_qk_proj(1, "k")
                make_upper_triangular(nc, tri[:, :], val=1.0, diag=True)
                nc.gpsimd.memset(v_sb[:, :, :, HD : HD + 1], 1.0)
                issue_xb_dma(0)
                issue_xb_dma(1)
                nc.scalar.dma_start(
                    out=bv_bc[:, :, :], in_=bvb.rearrange("p (h d) -> p h d", h=GH)
                )
                nc.scalar.dma_start(
                    out=wv_sb[:, :, :], in_=wvt.rearrange("(fc p) c -> p fc c", p=128)
                )
                prev_ctxn = None
                for qi in range(N_QC):
                    prev_ctxn = emit_attention(qi, prev_ctxn)
                for mq in range(4):
                    emit_phase3(4 * (N_QC - 1) + mq, tail=True)

    _split_multi_waits(nc)
    return nc


_CACHED = {}


def _build(reps=1):
    if reps not in _CACHED:
        nc = bass.Bass("TRN2", target_bir_lowering=False, debug=False)
        _CACHED[reps] = _emit_kernel(nc, reps)
    return _CACHED[reps]


def _perm_for_chunks():
    """c-dim permutation for the fp8 DoubleRow score layout.

    Chunk cc (128 W columns) covers head group hg=cc//2, row=cc%2:
    column p holds c = (4*hg + p//32)*64 + 32*row + (p%32)."""
    perm = np.empty(C, np.int64)
    for cc in range(NCC):
        hg, row = cc // 2, cc % 2
        p = np.arange(128)
        perm[cc * 128 : (cc + 1) * 128] = (4 * hg + p // 32) * 64 + 32 * row + (p % 32)
    return perm


_PERM = _perm_for_chunks()


def _reference_numpy(x, Wq, bq, Wk, bk, Wv, bv, Wo, bo, attention_mask):
    """Fallback for non-all-ones attention masks (spec fills ones)."""
    scale = HD ** -0.5
    out = np.empty((B, S, D), np.float32)
    causal = np.triu(np.ones((S, S), bool), k=1)
    for b in range(B):
        q = (x[b] @ Wq.T + bq).reshape(S, H, HD).transpose(1, 0, 2)
        k = (x[b] @ Wk.T + bk).reshape(S, H, HD).transpose(1, 0, 2)
        v = (x[b] @ Wv.T + bv).reshape(S, H, HD).transpose(1, 0, 2)
        o = np.empty((H, S, HD), np.float32)
        pad = (attention_mask[b] == 0)[None, :]
        for h in range(H):
            s = (q[h] @ k[h].T) * scale
            s[causal] = -np.inf
            s = np.where(pad, np.float32(-1e9), s)
            s -= s.max(-1, keepdims=True)
            e = np.exp(s)
            p = e / e.sum(-1, keepdims=True)
            o[h] = p @ v[h]
        ctx = o.transpose(1, 0, 2).reshape(S, D)
        out[b] = ctx @ Wo.T + bo
    return out


def kernel(x, Wq, bq, Wk, bk, Wv, bv, Wo, bo, attention_mask):
    x = np.asarray(x, np.float32)
    Wq, bq = np.asarray(Wq, np.float32), np.asarray(bq, np.float32)
    Wk, bk = np.asarray(Wk, np.float32), np.asarray(bk, np.float32)
    Wv, bv = np.asarray(Wv, np.float32), np.asarray(bv, np.float32)
    Wo, bo = np.asarray(Wo, np.float32), np.asarray(bo, np.float32)
    attention_mask = np.asarray(attention_mask)

    if not np.all(attention_mask == 1):
        return _reference_numpy(x, Wq, bq, Wk, bk, Wv, bv, Wo, bo, attention_mask)

    nc = _build()

    E4NP = ml_dtypes.float8_e4m3
    BFNP = ml_dtypes.bfloat16
    xbts = [np.ascontiguousarray(x[b].T.astype(BFNP)) for b in range(B)]
    x8ts = [np.ascontiguousarray(x[b].T.astype(E4NP)) for b in range(B)]
    shards = []
    for g in range(2):
        cs = slice(g * C, (g + 1) * C)
        Wq_c, Wk_c = Wq[cs, :][_PERM], Wk[cs, :][_PERM]
        bq_c, bk_c = bq[cs][_PERM], bk[cs][_PERM]
        shards.append(
            dict(
                wq8=np.ascontiguousarray((Wq_c * W_SCALE).T).astype(E4NP),
                wk8=np.ascontiguousarray((Wk_c * W_SCALE).T).astype(E4NP),
                wvt=np.ascontiguousarray(Wv[cs, :].T.astype(BFNP)),
                bqv=np.ascontiguousarray(
                    (bq_c * QK_STORE).reshape(NCC, 128).T
                ),
                bkv=np.ascontiguousarray(
                    (bk_c * QK_STORE).reshape(NCC, 128).T
                ),
                bvb=np.ascontiguousarray(np.broadcast_to(bv[cs], (128, C))),
                wot=np.ascontiguousarray(Wo[:, cs].T).astype(BFNP),
            )
        )
    in_maps = []
    for c in range(N_CORES):
        b, g = c // 2, c % 2
        in_maps.append(dict(xbt=xbts[b], x8t=x8ts[b], **shards[g]))

    res = run_bass_kernel_spmd(nc, in_maps, core_ids=list(range(N_CORES)))

    out = np.empty((B, S, D), np.float32)
    for b in range(B):
        out[b] = res.results[2 * b]["out"] + res.results[2 * b + 1]["out"] + bo
    return out


# revision 71
# speedup vs baseline: 1.0196x; 1.0196x over previous
# Multi-head attention (B=4, S=2048, D=1024, H=16) on 8 NeuronCores.
#
# Sharding: batch x head-group. Core c handles batch b=c//2 and heads
# 8*(c%2) .. 8*(c%2)+7 (a 512-wide slice of the model dim). Each core
# computes QKV projections for its slice, causal attention for its 8
# heads, and a row-parallel partial of the output projection. The host
# sums the two partials per batch and adds bo.
#
# Arithmetic / engine layout (v2, fp8 + flipped AV):
# - Q/K projections run as fp8e4m3 DoubleRow matmuls (2 contraction
#   k-tiles per instruction, 0.5 PE cycles/col): x8 = e4m3(x) and
#   W8 = e4m3(32 W) are quantized on the host. The bias add (DVE)
#   rescales PSUM by 1/2 and adds 16 b, storing 16(q+b) as fp8e4m3.
# - Scores also run as fp8 DoubleRow: the 64-dim head contraction is
#   split as 32 partitions x 2 rows. The W columns of Wq/Wk are permuted
#   host-side so each head's dims land as [32 partitions, 2 rows]. The
#   score PSUM is 256x the true scores; exp folds SCALE/256 into the
#   activation scale.
# - The AV matmul is flipped vs the baseline: out[q-part, c-free] with
#   et (bf16 probs) stationary and v (bf16, 64 cols + 1 ones-col for the
#   softmax denominator) moving: cost is 65 cols/block instead of 512.
#   The denominator lands per-partition, so normalization is a [128,4]
#   DVE reciprocal + per-head DVE tensor_scalar_mul - no ACT work.
# - Normalized ctx [q, c] goes back to [c, q] via bf16 SBUF->SBUF DMA
#   crossbar transposes (no engine time) to feed the row-parallel output
#   projection (bf16, unchanged from baseline). V projection runs bf16.
# - Engine legality: GPSIMD/Pool may not touch PSUM, so every
#   PSUM-reading op (biases, norm, output copies) sits on DVE; Pool gets
#   the SBUF-only causal tri-mask multiplies.
# - exp stays on ACT - the bottleneck engine (~152us busy: 147k
#   elem/partition + a ~190ns access bubble per instruction). Emission
#   interleaves DMA issue order with compute (DMA semaphores are
#   queue-cumulative), lags AV two heads behind scores, and spreads
#   projections/output-projection chunks into PE stalls between heads.
#
# Causality: fully-masked k-blocks are skipped, the exp of diagonal
# groups is trimmed to the valid column range, and the in-block triangle
# is zeroed with one [128,128] upper-tri mask multiply per diag block.
# Timeline sim: 214.3us vs 275.0us baseline (ACT-bound; PE ~133us,
# DVE ~90us, Pool ~50us busy). V projections run as just-in-time
# carries at the head of the phase whose AV consumes them, so each
# phase boundary leaves PE nearly empty for the next scores; fill
# slots are capped at one output-projection or projection piece each,
# because the exp read-ahead is limited to 2 score groups (~2.1us) by
# PSUM, and any longer PE burst between heads idles ACT.

import sys

for _p in ("/opt/trn_rl_repo", "/root/.axon_site/_ro/trn_rl_repo"):
    if _p not in sys.path:
        sys.path.append(_p)

import ml_dtypes
import numpy as np

import concourse.bass as bass
import concourse.mybir as mybir
import concourse.tile as tile
from concourse.bass_utils import run_bass_kernel_spmd
from concourse.masks import make_upper_triangular, make_identity

B, S, D, H = 4, 2048, 1024, 16
HD = D // H            # 64
N_CORES = 8
GH = 8                 # heads per core
C = GH * HD            # 512 local model dims per core
SCALE = HD ** -0.5
F32 = mybir.dt.float32
F32R = mybir.dt.float32r
BF16 = mybir.dt.bfloat16
E4 = mybir.dt.float8e4

T_CHUNK = 256          # t-tile for QKV projections
N_TC = 2048 // T_CHUNK # 8 t-chunks
QC = 512               # q columns per attention chunk
KB = 128               # k rows per attention block
N_KB = S // KB         # 16
N_QC = S // QC         # 4
EXP_GROUP = 2          # k-blocks per batched exp (2 psum banks x 2 bufs)

NFC = D // 128         # 8 f-chunks of the projection contraction
NFP = NFC // 2         # 4 DoubleRow f-pairs
NCC = C // 128         # 4 c-chunks of the local model dim

# fp8 scaling: W8 = e4m3(32 W), x8 = e4m3(x)  =>  psum = 32 q_nb
# stored q̂ = 16(q+b) = psum * 0.5 + 16 b ; score psum = 256 s
W_SCALE = 32.0
QK_STORE = 16.0
PS_TO_STORE = QK_STORE / W_SCALE            # 0.5
EXP_SCALE = SCALE / (QK_STORE * QK_STORE)   # fold 1/256 into exp


def _split_multi_waits(nc):
    """walrus in this container accepts only one sync-wait per instruction.
    Hoist all but the last wait of any multi-wait instruction onto NoOps
    inserted just before it on the same engine (sequencers execute their
    queue in order, so chained single waits are equivalent)."""
    for f in nc.m.functions:
        for blk in f.blocks:
            new_insts = []
            for inst in blk.instructions:
                si = inst.sync_info
                if si is not None and si.on_wait and len(si.on_wait) > 1:
                    waits = list(si.on_wait)
                    for i, w in enumerate(waits[:-1]):
                        nop = mybir.InstNoOp(name=f"{inst.name}_sw{i}", ins=[], outs=[])
                        nop.engine = inst.engine
                        nop.sync_info = mybir.SyncInfo(on_wait=[w], on_update=[])
                        new_insts.append(nop)
                    si.on_wait = [waits[-1]]
                new_insts.append(inst)
            blk.instructions[:] = new_insts


def _emit_kernel(nc, reps=1):
    xbt = nc.dram_tensor("xbt", [D, S], BF16, kind="ExternalInput").ap()
    x8t = nc.dram_tensor("x8t", [D, S], E4, kind="ExternalInput").ap()
    wq8 = nc.dram_tensor("wq8", [D, C], E4, kind="ExternalInput").ap()
    wk8 = nc.dram_tensor("wk8", [D, C], E4, kind="ExternalInput").ap()
    wvt = nc.dram_tensor("wvt", [D, C], BF16, kind="ExternalInput").ap()
    bqv = nc.dram_tensor("bqv", [128, NCC], F32, kind="ExternalInput").ap()
    bkv = nc.dram_tensor("bkv", [128, NCC], F32, kind="ExternalInput").ap()
    bvb = nc.dram_tensor("bvb", [128, C], F32, kind="ExternalInput").ap()
    wot = nc.dram_tensor("wot", [C, D], BF16, kind="ExternalInput").ap()
    out = nc.dram_tensor("out", [S, D], F32, kind="ExternalOutput").ap()

    with tile.TileContext(nc) as tc:
        import contextlib

        ctx = contextlib.ExitStack()
        with ctx:
            consts = ctx.enter_context(tc.tile_pool(name="consts", bufs=1))
            wpool = ctx.enter_context(tc.tile_pool(name="wpool", bufs=1))
            qkv = ctx.enter_context(tc.tile_pool(name="qkv", bufs=1))
            xtp = ctx.enter_context(tc.tile_pool(name="xtp", bufs=4))
            x8p = ctx.enter_context(tc.tile_pool(name="x8p", bufs=4))
            etp = ctx.enter_context(tc.tile_pool(name="etp", bufs=3))
            ctxnp = ctx.enter_context(tc.tile_pool(name="ctxnp", bufs=3))
            ctxp = ctx.enter_context(tc.tile_pool(name="ctxp", bufs=1))
            smallp = ctx.enter_context(tc.tile_pool(name="smallp", bufs=6))
            outp = ctx.enter_context(tc.tile_pool(name="outp", bufs=5))

            ps_qkv = ctx.enter_context(
                tc.tile_pool(name="ps_qkv", bufs=2, space="PSUM")
            )
            ps_sc = ctx.enter_context(
                tc.tile_pool(name="ps_sc", bufs=2, space="PSUM")
            )
            ps_av = ctx.enter_context(
                tc.tile_pool(name="ps_av", bufs=2, space="PSUM")
            )

            # ---- constants (tiles only; mask gen is emitted after the
            # prologue weight DMAs so it never heads the Pool queue) ----------
            tri = consts.tile([128, 128], BF16)      # tri[p, c] = 1.0 iff p <= c

            bv_bc = consts.tile([128, GH, HD], F32)  # bv broadcast across partitions

            bq_sb = consts.tile([128, NCC], F32)     # 16*bq[perm] at [p, cc]
            bk_sb = consts.tile([128, NCC], F32)

            # ---- weights + early input chunks ------------------------------
            # DMA semaphores are queue-cumulative (a consumer waits for
            # everything issued earlier on its queue), so DMAs are issued
            # interleaved with the compute that consumes them, in strict
            # first-need order per queue.
            wq_sb = wpool.tile([128, NFC, C], E4)
            wk_sb = wpool.tile([128, NFC, C], E4)
            wv_sb = wpool.tile([128, NFC, C], BF16)
            wo_sb = wpool.tile([128, NCC, D], BF16)

            _pref = {}

            def issue_x8_dma(tci):
                t0 = tci * T_CHUNK
                x8_c = x8p.tile([128, NFC, T_CHUNK], E4, name="x8_c")
                nc.sync.dma_start(
                    out=x8_c[:, :, :],
                    in_=x8t.rearrange("(fc p) t -> p fc t", p=128)[:, :, t0 : t0 + T_CHUNK],
                )
                _pref[tci] = (None, x8_c)

            def issue_xb_dma(tci):
                t0 = tci * T_CHUNK
                xb_c = xtp.tile([128, NFC, T_CHUNK], BF16, name="xb_c")
                nc.sync.dma_start(
                    out=xb_c[:, :, :],
                    in_=xbt.rearrange("(fc p) t -> p fc t", p=128)[:, :, t0 : t0 + T_CHUNK],
                )
                _pref[tci] = (xb_c, _pref[tci][1])

            def issue_chunk_dmas(tci):
                issue_x8_dma(tci)
                issue_xb_dma(tci)

            # ---- persistent activations -----------------------------------
            qt_sb = qkv.tile([128, NCC, S], E4)      # q̂: [32p x 2row per head]
            kt_sb = qkv.tile([128, NCC, S], E4)
            v_sb = qkv.tile([128, N_KB, GH, HD + 1], BF16)  # v + ones col
            ctx_sb = ctxp.tile([128, NCC, S], BF16)  # ctxT: [c within chunk, cc, q]

            def emit_qk_proj(tci, which, bias_engine="vector"):
                t0 = tci * T_CHUNK
                x8_c = _pref[tci][1]
                w_sb, b_sb, y_sb = (
                    (wq_sb, bq_sb, qt_sb) if which == "q" else (wk_sb, bk_sb, kt_sb)
                )
                for cc in range(NCC):
                    ps = ps_qkv.tile(
                        [128, T_CHUNK], F32, name=f"ps_{which}", tag="ps_qkv"
                    )
                    for i in range(NFP):
                        nc.tensor.matmul(
                            ps[:, :],
                            w_sb[:, 2 * i : 2 * i + 2, cc * 128 : (cc + 1) * 128],
                            x8_c[:, 2 * i : 2 * i + 2, :],
                            start=(i == 0),
                            stop=(i == NFP - 1),
                            perf_mode=mybir.MatmulPerfMode.DoubleRow,
                        )
                    if bias_engine == "scalar":
                        # prologue only: ACT is idle before the first exp,
                        # and the serial bias chain is the critical path
                        # to the first scores
                        nc.scalar.activation(
                            y_sb[:, cc, t0 : t0 + T_CHUNK],
                            ps[:, :],
                            mybir.ActivationFunctionType.Identity,
                            bias=b_sb[:, cc : cc + 1],
                            scale=PS_TO_STORE,
                        )
                    else:
                        eng = nc.vector if bias_engine == "vector" else nc.gpsimd
                        eng.tensor_scalar(
                            out=y_sb[:, cc, t0 : t0 + T_CHUNK],
                            in0=ps[:, :],
                            scalar1=PS_TO_STORE,
                            scalar2=b_sb[:, cc : cc + 1],
                            op0=mybir.AluOpType.mult,
                            op1=mybir.AluOpType.add,
                        )

            def emit_v_proj(tci, tt):
                t0 = tci * T_CHUNK
                xb_c = _pref[tci][0]
                kb = (t0 + tt * 128) // KB
                ps = ps_qkv.tile([128, C], F32, name="ps_v", tag="ps_qkv")
                for fc in range(NFC):
                    nc.tensor.matmul(
                        ps[:, :],
                        xb_c[:, fc, tt * 128 : (tt + 1) * 128],
                        wv_sb[:, fc, :],
                        start=(fc == 0),
                        stop=(fc == NFC - 1),
                    )
                nc.vector.tensor_add(
                    v_sb[:, kb, :, 0:HD],
                    ps.rearrange("p (h d) -> p h d", h=GH),
                    bv_bc[:, :, :],
                )

            def emit_scores(h, qi):
                hp = 32 * (h % 4)        # partition offset of this head
                hc = 2 * (h // 4)        # first of the head's 2 cc rows
                q0 = qi * QC
                nkb = 4 * qi + 4         # causal: k-blocks 0 .. 4qi+3
                et = etp.tile([128, N_KB, QC], BF16, name="et")
                n_grp = (nkb + EXP_GROUP - 1) // EXP_GROUP
                for gi in range(n_grp):
                    kb_lo = gi * EXP_GROUP
                    kb_hi = min(kb_lo + EXP_GROUP, nkb)
                    gw = kb_hi - kb_lo
                    sc_ps = ps_sc.tile([128, EXP_GROUP, QC], F32)
                    for kb in range(kb_lo, kb_hi):
                        nc.tensor.matmul(
                            sc_ps[:, kb - kb_lo, :],
                            kt_sb[hp : hp + 32, hc : hc + 2, kb * KB : (kb + 1) * KB],
                            qt_sb[hp : hp + 32, hc : hc + 2, q0 : q0 + QC],
                            start=True,
                            stop=True,
                            perf_mode=mybir.MatmulPerfMode.DoubleRow,
                            tile_position=(hp, 0),
                        )
                    # cols < 128*m of diagonal block m are never read by
                    # AV; a rectangular trim to the group's min offset is
                    # safe and cuts ACT work on the causal tail.
                    g_min_m = kb_lo - 4 * qi
                    g_off = 128 * g_min_m if g_min_m > 0 else 0
                    nc.scalar.activation(
                        et[:, kb_lo:kb_hi, g_off:QC],
                        sc_ps[:, 0:gw, g_off:QC],
                        mybir.ActivationFunctionType.Exp,
                        bias=0.0,
                        scale=EXP_SCALE,
                    )
                    for kb in range(kb_lo, kb_hi):
                        m = kb - 4 * qi  # >= 0 on the causal diagonal
                        if m >= 0:
                            # SBUF-only op: Pool can take it (it may not
                            # touch PSUM), keeping DVE for the PSUM readers
                            nc.gpsimd.tensor_mul(
                                et[:, kb, 128 * m : 128 * m + 128],
                                et[:, kb, 128 * m : 128 * m + 128],
                                tri[:, :],
                            )
                return et

            def emit_av_norm(h, qi, et, ctxn):
                # flipped AV: out[q-part, c-free], v moving (64 + ones col)
                av_ps = ps_av.tile([128, 4, HD + 1], F32, name="av", tag="ps_av")
                for mq in range(4):
                    qb = 4 * qi + mq
                    for kb in range(qb + 1):
                        nc.tensor.matmul(
                            av_ps[:, mq, :],
                            et[:, kb, 128 * mq : 128 * mq + 128],
                            v_sb[:, kb, h, :],
                            start=(kb == 0),
                            stop=(kb == qb),
                        )
                den = smallp.tile([128, 4], F32, name="den")
                nc.vector.tensor_copy(den[:, :], av_ps[:, :, HD])
                rec = smallp.tile([128, 4], F32, name="rec")
                nc.vector.reciprocal(rec[:, :], den[:, :])
                for mq in range(4):
                    nc.vector.tensor_scalar_mul(
                        ctxn[:, mq, h, :],
                        av_ps[:, mq, 0:HD],
                        rec[:, mq : mq + 1],
                    )

            def emit_transposes(qi, ctxn, mqs=range(4), hpairs=range(NCC)):
                # bf16 SBUF->SBUF transpose on the DMA crossbar: no engine
                # time at all (the sync queue carries the descriptors)
                for mq in mqs:
                    q0 = (4 * qi + mq) * 128
                    for hpair in hpairs:
                        nc.sync.dma_start_transpose(
                            ctx_sb[:, hpair, q0 : q0 + 128],
                            ctxn[:, mq, 2 * hpair : 2 * hpair + 2, :],
                        )

            def emit_phase3(qq, tail=False):
                for eh in range(2):
                    ps = ps_qkv.tile([128, D // 2], F32, name="ps_op", tag="ps_qkv")
                    for cc in range(NCC):
                        nc.tensor.matmul(
                            ps[:, :],
                            ctx_sb[:, cc, qq * 128 : (qq + 1) * 128],
                            wo_sb[:, cc, eh * (D // 2) : (eh + 1) * (D // 2)],
                            start=(cc == 0),
                            stop=(cc == NCC - 1),
                        )
                    o_sb = outp.tile([128, D // 2], F32, name="o_sb")
                    nc.vector.tensor_copy(o_sb[:, :], ps[:, :])
                    # at the drain tail spread the final stores across two
                    # queues (ACT is idle then); mid-stream keep them off
                    # the scalar queue so they never gate a chunk DMA
                    dma_q = nc.scalar if (tail and eh == 1) else nc.sync
                    dma_q.dma_start(
                        out=out[qq * 128 : (qq + 1) * 128, eh * (D // 2) : (eh + 1) * (D // 2)],
                        in_=o_sb[:, :],
                    )

            def emit_wo_dmas():
                for cc in range(NCC):
                    nc.scalar.dma_start(
                        out=wo_sb[:, cc, :], in_=wot[cc * 128 : (cc + 1) * 128, :]
                    )

            AV_LAG = 2  # AV trails scores by 2 heads (et pool bufs = LAG+1)

            def emit_attention(qi, prev_ctxn):
                """Heads of q-chunk qi with PE filler work interleaved.
                Scores(h) go first each slot (they feed ACT, the
                bottleneck); AV+norm lag AV_LAG heads so the V
                projections each AV needs are already queued; the
                transposes + output projection of qi-1 and the
                projections for qi+1 fill PE stalls between heads."""
                # V projections for THIS qi's new k-blocks run as a
                # just-in-time carry at h1/h2 (before AV(h0) at the lag
                # slot), so the end of each phase leaves PE nearly empty
                # and the next phase's first scores issue immediately.
                t_a, t_b = 2 * qi + 2, 2 * qi + 3  # next qi's chunks
                op0 = 4 * (qi - 1)
                if qi == 0:
                    fill = [
                        [lambda: emit_v_proj(0, 0), lambda: emit_v_proj(0, 1)],
                        [lambda: emit_v_proj(1, 0), lambda: emit_v_proj(1, 1)],
                        [lambda: issue_chunk_dmas(t_a),
                         lambda: issue_chunk_dmas(t_b),
                         emit_wo_dmas],
                        [lambda: emit_qk_proj(t_a, "q")],
                        [lambda: emit_qk_proj(t_a, "k")],
                        [lambda: emit_qk_proj(t_b, "q")],
                        [lambda: emit_qk_proj(t_b, "k")],
                        [lambda: issue_chunk_dmas(t_a + 2),
                         lambda: issue_chunk_dmas(t_b + 2)],
                    ]
                elif qi < N_QC - 1:
                    more = 2 * qi + 4 < N_TC
                    fill = [
                        [lambda: emit_transposes(qi - 1, prev_ctxn, (0, 1)),
                         lambda: emit_v_proj(2 * qi, 0),
                         lambda: emit_v_proj(2 * qi, 1)],
                        [lambda: emit_transposes(qi - 1, prev_ctxn, (2, 3)),
                         lambda: emit_v_proj(2 * qi + 1, 0),
                         lambda: emit_v_proj(2 * qi + 1, 1)],
                        [lambda: emit_phase3(op0)],
                        [lambda: emit_phase3(op0 + 1), lambda: emit_qk_proj(t_a, "q")],
                        [lambda: emit_phase3(op0 + 2), lambda: emit_qk_proj(t_a, "k")],
                        [lambda: emit_phase3(op0 + 3), lambda: emit_qk_proj(t_b, "q")],
                        [lambda: emit_qk_proj(t_b, "k")],
                        ([lambda: issue_chunk_dmas(2 * qi + 4),
                          lambda: issue_chunk_dmas(2 * qi + 5)] if more else []),
                    ]
                else:
                    fill = [
                        [lambda: emit_transposes(qi - 1, prev_ctxn, (0, 1)),
                         lambda: emit_v_proj(2 * qi, 0),
                         lambda: emit_v_proj(2 * qi, 1)],
                        [lambda: emit_transposes(qi - 1, prev_ctxn, (2, 3)),
                         lambda: emit_v_proj(2 * qi + 1, 0),
                         lambda: emit_v_proj(2 * qi + 1, 1)],
                        [lambda: emit_phase3(op0)],
                        [lambda: emit_phase3(op0 + 1)],
                        [lambda: emit_phase3(op0 + 2)],
                        [lambda: emit_phase3(op0 + 3)],
                    ]

                # transposes chase completed head pairs on the last qi
                last = qi == N_QC - 1
                lag = AV_LAG
                ctxn = ctxnp.tile([128, 4, GH, HD], BF16, name="ctxn", tag="ctxn")

                def after_norm(hn):
                    if last and hn % 2 == 1:
                        emit_transposes(qi, ctxn, hpairs=(hn // 2,))

                ets = {}
                for h in range(GH):
                    ets[h] = emit_scores(h, qi)
                    if h >= 1 and h - 1 < len(fill):
                        for f in fill[h - 1]:
                            f()
                    if h >= lag:
                        emit_av_norm(h - lag, qi, ets.pop(h - lag), ctxn)
                        after_norm(h - lag)
                for slot in fill[GH - 1 :]:
                    for f in slot:
                        f()
                for h in range(GH - lag, GH):
                    emit_av_norm(h, qi, ets.pop(h), ctxn)
                    after_norm(h)
                return ctxn

            for _rep in range(reps):
                # critical path first: x8(t0,t1), wq8, bq -> q projections;
                # then the k side; mask gen and the V/O weight queue follow.
                issue_x8_dma(0)
                issue_x8_dma(1)
                nc.gpsimd.dma_start(
                    out=wq_sb[:, :, :], in_=wq8.rearrange("(fc p) c -> p fc c", p=128)
                )
                nc.scalar.dma_start(out=bq_sb[:, :], in_=bqv)
                emit_qk_proj(0, "q")
                emit_qk_proj(1, "q")
                nc.gpsimd.dma_start(
                    out=wk_sb[:, :, :], in_=wk8.rearrange("(fc p) c -> p fc c", p=128)
                )
                nc.scalar.dma_start(out=bk_sb[:, :], in_=bkv)
                emit_qk_proj(0, "k")
                emit_qk_proj(1, "k")
                make_upper_triangular(nc, tri[:, :], val=1.0, diag=True)
                nc.gpsimd.memset(v_sb[:, :, :, HD : HD + 1], 1.0)
                issue_xb_dma(0)
                issue_xb_dma(1)
                nc.scalar.dma_start(
                    out=bv_bc[:, :, :], in_=bvb.rearrange("p (h d) -> p h d", h=GH)
                )
                nc.scalar.dma_start(
                    out=wv_sb[:, :, :], in_=wvt.rearrange("(fc p) c -> p fc c", p=128)
                )
                prev_ctxn = None
                for qi in range(N_QC):
                    prev_ctxn = emit_attention(qi, prev_ctxn)
                for mq in range(4):
                    emit_phase3(4 * (N_QC - 1) + mq, tail=True)

    _split_multi_waits(nc)
    return nc


_CACHED = {}


def _build(reps=1):
    if reps not in _CACHED:
        nc = bass.Bass("TRN2", target_bir_lowering=False, debug=False)
        _CACHED[reps] = _emit_kernel(nc, reps)
    return _CACHED[reps]


def _perm_for_chunks():
    """c-dim permutation for the fp8 DoubleRow score layout.

    Chunk cc (128 W columns) covers head group hg=cc//2, row=cc%2:
    column p holds c = (4*hg + p//32)*64 + 32*row + (p%32)."""
    perm = np.empty(C, np.int64)
    for cc in range(NCC):
        hg, row = cc // 2, cc % 2
        p = np.arange(128)
        perm[cc * 128 : (cc + 1) * 128] = (4 * hg + p // 32) * 64 + 32 * row + (p % 32)
    return perm


_PERM = _perm_for_chunks()


def _reference_numpy(x, Wq, bq, Wk, bk, Wv, bv, Wo, bo, attention_mask):
    """Fallback for non-all-ones attention masks (spec fills ones)."""
    scale = HD ** -0.5
    out = np.empty((B, S, D), np.float32)
    causal = np.triu(np.ones((S, S), bool), k=1)
    for b in range(B):
        q = (x[b] @ Wq.T + bq).reshape(S, H, HD).transpose(1, 0, 2)
        k = (x[b] @ Wk.T + bk).reshape(S, H, HD).transpose(1, 0, 2)
        v = (x[b] @ Wv.T + bv).reshape(S, H, HD).transpose(1, 0, 2)
        o = np.empty((H, S, HD), np.float32)
        pad = (attention_mask[b] == 0)[None, :]
        for h in range(H):
            s = (q[h] @ k[h].T) * scale
            s[causal] = -np.inf
            s = np.where(pad, np.float32(-1e9), s)
            s -= s.max(-1, keepdims=True)
            e = np.exp(s)
            p = e / e.sum(-1, keepdims=True)
            o[h] = p @ v[h]
        ctx = o.transpose(1, 0, 2).reshape(S, D)
        out[b] = ctx @ Wo.T + bo
    return out


def kernel(x, Wq, bq, Wk, bk, Wv, bv, Wo, bo, attention_mask):
    x = np.asarray(x, np.float32)
    Wq, bq = np.asarray(Wq, np.float32), np.asarray(bq, np.float32)
    Wk, bk = np.asarray(Wk, np.float32), np.asarray(bk, np.float32)
    Wv, bv = np.asarray(Wv, np.float32), np.asarray(bv, np.float32)
    Wo, bo = np.asarray(Wo, np.float32), np.asarray(bo, np.float32)
    attention_mask = np.asarray(attention_mask)

    if not np.all(attention_mask == 1):
        return _reference_numpy(x, Wq, bq, Wk, bk, Wv, bv, Wo, bo, attention_mask)

    nc = _build()

    E4NP = ml_dtypes.float8_e4m3
    BFNP = ml_dtypes.bfloat16
    xbts = [np.ascontiguousarray(x[b].T.astype(BFNP)) for b in range(B)]
    x8ts = [np.ascontiguousarray(x[b].T.astype(E4NP)) for b in range(B)]
    shards = []
    for g in range(2):
        cs = slice(g * C, (g + 1) * C)
        Wq_c, Wk_c = Wq[cs, :][_PERM], Wk[cs, :][_PERM]
        bq_c, bk_c = bq[cs][_PERM], bk[cs][_PERM]
        shards.append(
            dict(
                wq8=np.ascontiguousarray((Wq_c * W_SCALE).T).astype(E4NP),
                wk8=np.ascontiguousarray((Wk_c * W_SCALE).T).astype(E4NP),
                wvt=np.ascontiguousarray(Wv[cs, :].T.astype(BFNP)),
                bqv=np.ascontiguousarray(
                    (bq_c * QK_STORE).reshape(NCC, 128).T
                ),
                bkv=np.ascontiguousarray(
                    (bk_c * QK_STORE).reshape(NCC, 128).T
                ),
                bvb=np.ascontiguousarray(np.broadcast_to(bv[cs], (128, C))),
                wot=np.ascontiguousarray(Wo[:, cs].T).astype(BFNP),
            )
        )
    in_maps = []
    for c in range(N_CORES):
        b, g = c // 2, c % 2
        in_maps.append(dict(xbt=xbts[b], x8t=x8ts[b], **shards[g]))

    res = run_bass_kernel_spmd(nc, in_maps, core_ids=list(range(N_CORES)))

    out = np.empty((B, S, D), np.float32)
    for b in range(B):
        out[b] = res.results[2 * b]["out"] + res.results[2 * b + 1]["out"] + bo
    return out


# revision 73
# speedup vs baseline: 1.0420x; 1.0219x over previous
# Multi-head attention (B=4, S=2048, D=1024, H=16) on 8 NeuronCores.
#
# Sharding: batch x head-group. Core c handles batch b=c//2 and heads
# 8*(c%2) .. 8*(c%2)+7 (a 512-wide slice of the model dim). Each core
# computes QKV projections for its slice, causal attention for its 8
# heads, and a row-parallel partial of the output projection. The host
# sums the two partials per batch and adds bo.
#
# Arithmetic / engine layout (v2, fp8 + flipped AV):
# - Q/K projections run as fp8e4m3 DoubleRow matmuls (2 contraction
#   k-tiles per instruction, 0.5 PE cycles/col): x8 = e4m3(x) and
#   W8 = e4m3(32 W) are quantized on the host. The bias add (DVE)
#   rescales PSUM by 1/2 and adds 16 b, storing 16(q+b) as fp8e4m3.
# - Scores also run as fp8 DoubleRow: the 64-dim head contraction is
#   split as 32 partitions x 2 rows. The W columns of Wq/Wk are permuted
#   host-side so each head's dims land as [32 partitions, 2 rows]. The
#   score PSUM is 256x the true scores; exp folds SCALE/256 into the
#   activation scale.
# - The AV matmul is flipped vs the baseline: out[q-part, c-free] with
#   et (bf16 probs) stationary and v (bf16, 64 cols + 1 ones-col for the
#   softmax denominator) moving: cost is 65 cols/block instead of 512.
#   The denominator lands per-partition, so normalization is a [128,4]
#   DVE reciprocal + per-head DVE tensor_scalar_mul - no ACT work.
# - Normalized ctx [q, c] goes back to [c, q] via bf16 SBUF->SBUF DMA
#   crossbar transposes (no engine time) to feed the row-parallel output
#   projection (bf16, unchanged from baseline). V projection runs bf16.
# - Engine legality: GPSIMD/Pool may not touch PSUM, so every
#   PSUM-reading op (biases, norm, output copies) sits on DVE; Pool gets
#   the SBUF-only causal tri-mask multiplies.
# - exp stays on ACT - the bottleneck engine (~152us busy: 147k
#   elem/partition + a ~190ns access bubble per instruction). Emission
#   interleaves DMA issue order with compute (DMA semaphores are
#   queue-cumulative), lags AV two heads behind scores, and spreads
#   projections/output-projection chunks into PE stalls between heads.
#
# Causality: fully-masked k-blocks are skipped, the exp of diagonal
# groups is trimmed to the valid column range, and the in-block triangle
# is zeroed with one [128,128] upper-tri mask multiply per diag block.
# Timeline sim: 210.2us vs 275.0us baseline (ACT-bound; PE ~133us,
# DVE ~90us, Pool ~50us busy). V projections run as just-in-time
# carries at the head of the phase whose AV consumes them, so each
# phase boundary leaves PE nearly empty for the next scores; fill
# slots are capped at one output-projection or projection piece each,
# because the exp read-ahead is limited to 2 score groups (~2.1us) by
# PSUM, and any longer PE burst between heads idles ACT.

import sys

for _p in ("/opt/trn_rl_repo", "/root/.axon_site/_ro/trn_rl_repo"):
    if _p not in sys.path:
        sys.path.append(_p)

import ml_dtypes
import numpy as np

import concourse.bass as bass
import concourse.mybir as mybir
import concourse.tile as tile
from concourse.bass_utils import run_bass_kernel_spmd
from concourse.masks import make_upper_triangular, make_identity

B, S, D, H = 4, 2048, 1024, 16
HD = D // H            # 64
N_CORES = 8
GH = 8                 # heads per core
C = GH * HD            # 512 local model dims per core
SCALE = HD ** -0.5
F32 = mybir.dt.float32
F32R = mybir.dt.float32r
BF16 = mybir.dt.bfloat16
E4 = mybir.dt.float8e4

T_CHUNK = 256          # t-tile for QKV projections
N_TC = 2048 // T_CHUNK # 8 t-chunks
QC = 512               # q columns per attention chunk
KB = 128               # k rows per attention block
N_KB = S // KB         # 16
N_QC = S // QC         # 4
EXP_GROUP = 2          # k-blocks per batched exp (2 psum banks x 2 bufs)

NFC = D // 128         # 8 f-chunks of the projection contraction
NFP = NFC // 2         # 4 DoubleRow f-pairs
NCC = C // 128         # 4 c-chunks of the local model dim

# fp8 scaling: W8 = e4m3(32 W), x8 = e4m3(x)  =>  psum = 32 q_nb
# stored q̂ = 16(q+b) = psum * 0.5 + 16 b ; score psum = 256 s
W_SCALE = 32.0
QK_STORE = 16.0
PS_TO_STORE = QK_STORE / W_SCALE            # 0.5
EXP_SCALE = SCALE / (QK_STORE * QK_STORE)   # fold 1/256 into exp


def _split_multi_waits(nc):
    """walrus in this container accepts only one sync-wait per instruction.
    Hoist all but the last wait of any multi-wait instruction onto NoOps
    inserted just before it on the same engine (sequencers execute their
    queue in order, so chained single waits are equivalent)."""
    for f in nc.m.functions:
        for blk in f.blocks:
            new_insts = []
            for inst in blk.instructions:
                si = inst.sync_info
                if si is not None and si.on_wait and len(si.on_wait) > 1:
                    waits = list(si.on_wait)
                    for i, w in enumerate(waits[:-1]):
                        nop = mybir.InstNoOp(name=f"{inst.name}_sw{i}", ins=[], outs=[])
                        nop.engine = inst.engine
                        nop.sync_info = mybir.SyncInfo(on_wait=[w], on_update=[])
                        new_insts.append(nop)
                    si.on_wait = [waits[-1]]
                new_insts.append(inst)
            blk.instructions[:] = new_insts


def _emit_kernel(nc, reps=1):
    xbt = nc.dram_tensor("xbt", [D, S], BF16, kind="ExternalInput").ap()
    x8t = nc.dram_tensor("x8t", [D, S], E4, kind="ExternalInput").ap()
    wq8 = nc.dram_tensor("wq8", [D, C], E4, kind="ExternalInput").ap()
    wk8 = nc.dram_tensor("wk8", [D, C], E4, kind="ExternalInput").ap()
    wvt = nc.dram_tensor("wvt", [D, C], BF16, kind="ExternalInput").ap()
    bqv = nc.dram_tensor("bqv", [128, NCC], F32, kind="ExternalInput").ap()
    bkv = nc.dram_tensor("bkv", [128, NCC], F32, kind="ExternalInput").ap()
    bvb = nc.dram_tensor("bvb", [128, C], F32, kind="ExternalInput").ap()
    wot = nc.dram_tensor("wot", [C, D], BF16, kind="ExternalInput").ap()
    out = nc.dram_tensor("out", [S, D], F32, kind="ExternalOutput").ap()

    with tile.TileContext(nc) as tc:
        import contextlib

        ctx = contextlib.ExitStack()
        with ctx:
            consts = ctx.enter_context(tc.tile_pool(name="consts", bufs=1))
            wpool = ctx.enter_context(tc.tile_pool(name="wpool", bufs=1))
            qkv = ctx.enter_context(tc.tile_pool(name="qkv", bufs=1))
            xtp = ctx.enter_context(tc.tile_pool(name="xtp", bufs=4))
            x8p = ctx.enter_context(tc.tile_pool(name="x8p", bufs=5))
            etp = ctx.enter_context(tc.tile_pool(name="etp", bufs=3))
            ctxnp = ctx.enter_context(tc.tile_pool(name="ctxnp", bufs=3))
            ctxp = ctx.enter_context(tc.tile_pool(name="ctxp", bufs=1))
            smallp = ctx.enter_context(tc.tile_pool(name="smallp", bufs=8))
            outp = ctx.enter_context(tc.tile_pool(name="outp", bufs=6))

            ps_qkv = ctx.enter_context(
                tc.tile_pool(name="ps_qkv", bufs=2, space="PSUM")
            )
            ps_sc = ctx.enter_context(
                tc.tile_pool(name="ps_sc", bufs=2, space="PSUM")
            )
            ps_av = ctx.enter_context(
                tc.tile_pool(name="ps_av", bufs=2, space="PSUM")
            )

            # ---- constants (tiles only; mask gen is emitted after the
            # prologue weight DMAs so it never heads the Pool queue) ----------
            tri = consts.tile([128, 128], BF16)      # tri[p, c] = 1.0 iff p <= c

            bv_bc = consts.tile([128, GH, HD], F32)  # bv broadcast across partitions

            bq_sb = consts.tile([128, NCC], F32)     # 16*bq[perm] at [p, cc]
            bk_sb = consts.tile([128, NCC], F32)

            # ---- weights + early input chunks ------------------------------
            # DMA semaphores are queue-cumulative (a consumer waits for
            # everything issued earlier on its queue), so DMAs are issued
            # interleaved with the compute that consumes them, in strict
            # first-need order per queue.
            wq_sb = wpool.tile([128, NFC, C], E4)
            wk_sb = wpool.tile([128, NFC, C], E4)
            wv_sb = wpool.tile([128, NFC, C], BF16)
            wo_sb = wpool.tile([128, NCC, D], BF16)

            _pref = {}

            def issue_x8_dma(tci):
                t0 = tci * T_CHUNK
                x8_c = x8p.tile([128, NFC, T_CHUNK], E4, name="x8_c")
                nc.sync.dma_start(
                    out=x8_c[:, :, :],
                    in_=x8t.rearrange("(fc p) t -> p fc t", p=128)[:, :, t0 : t0 + T_CHUNK],
                )
                _pref[tci] = (None, x8_c)

            def issue_xb_dma(tci):
                t0 = tci * T_CHUNK
                xb_c = xtp.tile([128, NFC, T_CHUNK], BF16, name="xb_c")
                nc.sync.dma_start(
                    out=xb_c[:, :, :],
                    in_=xbt.rearrange("(fc p) t -> p fc t", p=128)[:, :, t0 : t0 + T_CHUNK],
                )
                _pref[tci] = (xb_c, _pref[tci][1])

            def issue_chunk_dmas(tci):
                issue_x8_dma(tci)
                issue_xb_dma(tci)

            # ---- persistent activations -----------------------------------
            qt_sb = qkv.tile([128, NCC, S], E4)      # q̂: [32p x 2row per head]
            kt_sb = qkv.tile([128, NCC, S], E4)
            v_sb = qkv.tile([128, N_KB, GH, HD + 1], BF16)  # v + ones col
            ctx_sb = ctxp.tile([128, NCC, S], BF16)  # ctxT: [c within chunk, cc, q]

            def emit_qk_proj(tci, which, bias_engine="vector"):
                t0 = tci * T_CHUNK
                x8_c = _pref[tci][1]
                w_sb, b_sb, y_sb = (
                    (wq_sb, bq_sb, qt_sb) if which == "q" else (wk_sb, bk_sb, kt_sb)
                )
                for cc in range(NCC):
                    ps = ps_qkv.tile(
                        [128, T_CHUNK], F32, name=f"ps_{which}", tag="ps_qkv"
                    )
                    for i in range(NFP):
                        nc.tensor.matmul(
                            ps[:, :],
                            w_sb[:, 2 * i : 2 * i + 2, cc * 128 : (cc + 1) * 128],
                            x8_c[:, 2 * i : 2 * i + 2, :],
                            start=(i == 0),
                            stop=(i == NFP - 1),
                            perf_mode=mybir.MatmulPerfMode.DoubleRow,
                        )
                    if bias_engine == "scalar":
                        # prologue only: ACT is idle before the first exp,
                        # and the serial bias chain is the critical path
                        # to the first scores
                        nc.scalar.activation(
                            y_sb[:, cc, t0 : t0 + T_CHUNK],
                            ps[:, :],
                            mybir.ActivationFunctionType.Identity,
                            bias=b_sb[:, cc : cc + 1],
                            scale=PS_TO_STORE,
                        )
                    else:
                        eng = nc.vector if bias_engine == "vector" else nc.gpsimd
                        eng.tensor_scalar(
                            out=y_sb[:, cc, t0 : t0 + T_CHUNK],
                            in0=ps[:, :],
                            scalar1=PS_TO_STORE,
                            scalar2=b_sb[:, cc : cc + 1],
                            op0=mybir.AluOpType.mult,
                            op1=mybir.AluOpType.add,
                        )

            def emit_v_proj(tci, tt):
                t0 = tci * T_CHUNK
                xb_c = _pref[tci][0]
                kb = (t0 + tt * 128) // KB
                ps = ps_qkv.tile([128, C], F32, name="ps_v", tag="ps_qkv")
                for fc in range(NFC):
                    nc.tensor.matmul(
                        ps[:, :],
                        xb_c[:, fc, tt * 128 : (tt + 1) * 128],
                        wv_sb[:, fc, :],
                        start=(fc == 0),
                        stop=(fc == NFC - 1),
                    )
                nc.vector.tensor_add(
                    v_sb[:, kb, :, 0:HD],
                    ps.rearrange("p (h d) -> p h d", h=GH),
                    bv_bc[:, :, :],
                )

            def emit_scores(h, qi):
                hp = 32 * (h % 4)        # partition offset of this head
                hc = 2 * (h // 4)        # first of the head's 2 cc rows
                q0 = qi * QC
                nkb = 4 * qi + 4         # causal: k-blocks 0 .. 4qi+3
                et = etp.tile([128, N_KB, QC], BF16, name="et")
                n_grp = (nkb + EXP_GROUP - 1) // EXP_GROUP
                for gi in range(n_grp):
                    kb_lo = gi * EXP_GROUP
                    kb_hi = min(kb_lo + EXP_GROUP, nkb)
                    gw = kb_hi - kb_lo
                    sc_ps = ps_sc.tile([128, EXP_GROUP, QC], F32)
                    for kb in range(kb_lo, kb_hi):
                        nc.tensor.matmul(
                            sc_ps[:, kb - kb_lo, :],
                            kt_sb[hp : hp + 32, hc : hc + 2, kb * KB : (kb + 1) * KB],
                            qt_sb[hp : hp + 32, hc : hc + 2, q0 : q0 + QC],
                            start=True,
                            stop=True,
                            perf_mode=mybir.MatmulPerfMode.DoubleRow,
                            tile_position=(hp, 0),
                        )
                    # cols < 128*m of diagonal block m are never read by
                    # AV; a rectangular trim to the group's min offset is
                    # safe and cuts ACT work on the causal tail.
                    g_min_m = kb_lo - 4 * qi
                    g_off = 128 * g_min_m if g_min_m > 0 else 0
                    nc.scalar.activation(
                        et[:, kb_lo:kb_hi, g_off:QC],
                        sc_ps[:, 0:gw, g_off:QC],
                        mybir.ActivationFunctionType.Exp,
                        bias=0.0,
                        scale=EXP_SCALE,
                    )
                    for kb in range(kb_lo, kb_hi):
                        m = kb - 4 * qi  # >= 0 on the causal diagonal
                        if m >= 0:
                            # SBUF-only op: Pool can take it (it may not
                            # touch PSUM), keeping DVE for the PSUM readers
                            nc.gpsimd.tensor_mul(
                                et[:, kb, 128 * m : 128 * m + 128],
                                et[:, kb, 128 * m : 128 * m + 128],
                                tri[:, :],
                            )
                return et

            def emit_av_norm(h, qi, et, ctxn):
                # flipped AV: out[q-part, c-free], v moving (64 + ones col)
                av_ps = ps_av.tile([128, 4, HD + 1], F32, name="av", tag="ps_av")
                for mq in range(4):
                    qb = 4 * qi + mq
                    for kb in range(qb + 1):
                        nc.tensor.matmul(
                            av_ps[:, mq, :],
                            et[:, kb, 128 * mq : 128 * mq + 128],
                            v_sb[:, kb, h, :],
                            start=(kb == 0),
                            stop=(kb == qb),
                        )
                den = smallp.tile([128, 4], F32, name="den")
                nc.vector.tensor_copy(den[:, :], av_ps[:, :, HD])
                rec = smallp.tile([128, 4], F32, name="rec")
                nc.vector.reciprocal(rec[:, :], den[:, :])
                for mq in range(4):
                    nc.vector.tensor_scalar_mul(
                        ctxn[:, mq, h, :],
                        av_ps[:, mq, 0:HD],
                        rec[:, mq : mq + 1],
                    )

            def emit_transposes(qi, ctxn, mqs=range(4), hpairs=range(NCC)):
                # bf16 SBUF->SBUF transpose on the DMA crossbar: no engine
                # time at all (the sync queue carries the descriptors)
                for mq in mqs:
                    q0 = (4 * qi + mq) * 128
                    for hpair in hpairs:
                        nc.sync.dma_start_transpose(
                            ctx_sb[:, hpair, q0 : q0 + 128],
                            ctxn[:, mq, 2 * hpair : 2 * hpair + 2, :],
                        )

            def emit_phase3(qq, tail=False):
                for eh in range(2):
                    ps = ps_qkv.tile([128, D // 2], F32, name="ps_op", tag="ps_qkv")
                    for cc in range(NCC):
                        nc.tensor.matmul(
                            ps[:, :],
                            ctx_sb[:, cc, qq * 128 : (qq + 1) * 128],
                            wo_sb[:, cc, eh * (D // 2) : (eh + 1) * (D // 2)],
                            start=(cc == 0),
                            stop=(cc == NCC - 1),
                        )
                    o_sb = outp.tile([128, D // 2], F32, name="o_sb")
                    nc.vector.tensor_copy(o_sb[:, :], ps[:, :])
                    # at the drain tail spread the final stores across two
                    # queues (ACT is idle then); mid-stream keep them off
                    # the scalar queue so they never gate a chunk DMA
                    dma_q = nc.scalar if (tail and eh == 1) else nc.sync
                    dma_q.dma_start(
                        out=out[qq * 128 : (qq + 1) * 128, eh * (D // 2) : (eh + 1) * (D // 2)],
                        in_=o_sb[:, :],
                    )

            def emit_wo_dmas():
                for cc in range(NCC):
                    nc.scalar.dma_start(
                        out=wo_sb[:, cc, :], in_=wot[cc * 128 : (cc + 1) * 128, :]
                    )

            AV_LAG = 2  # AV trails scores by 2 heads (et pool bufs = LAG+1)

            def emit_attention(qi, prev_ctxn):
                """Heads of q-chunk qi with PE filler work interleaved.
                Scores(h) go first each slot (they feed ACT, the
                bottleneck); AV+norm lag AV_LAG heads so the V
                projections each AV needs are already queued; the
                transposes + output projection of qi-1 and the
                projections for qi+1 fill PE stalls between heads."""
                # V projections for THIS qi's new k-blocks run as a
                # just-in-time carry at h1/h2 (before AV(h0) at the lag
                # slot), so the end of each phase leaves PE nearly empty
                # and the next phase's first scores issue immediately.
                t_a, t_b = 2 * qi + 2, 2 * qi + 3  # next qi's chunks
                op0 = 4 * (qi - 1)
                if qi == 0:
                    fill = [
                        [lambda: emit_v_proj(0, 0), lambda: emit_v_proj(0, 1)],
                        [lambda: emit_v_proj(1, 0), lambda: emit_v_proj(1, 1)],
                        [lambda: issue_chunk_dmas(t_a),
                         lambda: issue_chunk_dmas(t_b),
                         emit_wo_dmas],
                        [lambda: emit_qk_proj(t_a, "q")],
                        [lambda: emit_qk_proj(t_a, "k")],
                        [lambda: emit_qk_proj(t_b, "q")],
                        [lambda: emit_qk_proj(t_b, "k")],
                        [lambda: issue_chunk_dmas(t_a + 2),
                         lambda: issue_chunk_dmas(t_b + 2)],
                    ]
                elif qi < N_QC - 1:
                    more = 2 * qi + 4 < N_TC
                    fill = [
                        [lambda: emit_transposes(qi - 1, prev_ctxn, (0, 1)),
                         lambda: emit_v_proj(2 * qi, 0),
                         lambda: emit_v_proj(2 * qi, 1)],
                        [lambda: emit_transposes(qi - 1, prev_ctxn, (2, 3)),
                         lambda: emit_v_proj(2 * qi + 1, 0),
                         lambda: emit_v_proj(2 * qi + 1, 1)],
                        [lambda: emit_phase3(op0)],
                        [lambda: emit_phase3(op0 + 1), lambda: emit_qk_proj(t_a, "q")],
                        [lambda: emit_phase3(op0 + 2), lambda: emit_qk_proj(t_a, "k")],
                        [lambda: emit_phase3(op0 + 3), lambda: emit_qk_proj(t_b, "q")],
                        [lambda: emit_qk_proj(t_b, "k")],
                        ([lambda: issue_chunk_dmas(2 * qi + 4),
                          lambda: issue_chunk_dmas(2 * qi + 5)] if more else []),
                    ]
                else:
                    fill = [
                        [lambda: emit_transposes(qi - 1, prev_ctxn, (0, 1)),
                         lambda: emit_v_proj(2 * qi, 0),
                         lambda: emit_v_proj(2 * qi, 1)],
                        [lambda: emit_transposes(qi - 1, prev_ctxn, (2, 3)),
                         lambda: emit_v_proj(2 * qi + 1, 0),
                         lambda: emit_v_proj(2 * qi + 1, 1)],
                        [lambda: emit_phase3(op0)],
                        [lambda: emit_phase3(op0 + 1)],
                        [lambda: emit_phase3(op0 + 2)],
                        [lambda: emit_phase3(op0 + 3)],
                    ]

                # transposes chase completed head pairs on the last qi
                last = qi == N_QC - 1
                lag = AV_LAG
                ctxn = ctxnp.tile([128, 4, GH, HD], BF16, name="ctxn", tag="ctxn")

                def after_norm(hn):
                    if last and hn % 2 == 1:
                        emit_transposes(qi, ctxn, hpairs=(hn // 2,))

                ets = {}
                for h in range(GH):
                    ets[h] = emit_scores(h, qi)
                    if h >= 1 and h - 1 < len(fill):
                        for f in fill[h - 1]:
                            f()
                    if h >= lag:
                        emit_av_norm(h - lag, qi, ets.pop(h - lag), ctxn)
                        after_norm(h - lag)
                for slot in fill[GH - 1 :]:
                    for f in slot:
                        f()
                for h in range(GH - lag, GH):
                    emit_av_norm(h, qi, ets.pop(h), ctxn)
                    after_norm(h)
                return ctxn

            for _rep in range(reps):
                # critical path first: x8(t0,t1), wq8, bq -> q projections;
                # then the k side; mask gen and the V/O weight queue follow.
                issue_x8_dma(0)
                issue_x8_dma(1)
                nc.gpsimd.dma_start(
                    out=wq_sb[:, :, :], in_=wq8.rearrange("(fc p) c -> p fc c", p=128)
                )
                nc.scalar.dma_start(out=bq_sb[:, :], in_=bqv)
                emit_qk_proj(0, "q")
                emit_qk_proj(1, "q")
                nc.gpsimd.dma_start(
                    out=wk_sb[:, :, :], in_=wk8.rearrange("(fc p) c -> p fc c", p=128)
                )
                nc.scalar.dma_start(out=bk_sb[:, :], in_=bkv)
                emit_qk_proj(0, "k")
                emit_qk_proj(1, "k")
                make_upper_triangular(nc, tri[:, :], val=1.0, diag=True)
                nc.gpsimd.memset(v_sb[:, :, :, HD : HD + 1], 1.0)
                issue_xb_dma(0)
                issue_xb_dma(1)
                nc.scalar.dma_start(
                    out=bv_bc[:, :, :], in_=bvb.rearrange("p (h d) -> p h d", h=GH)
                )
                nc.scalar.dma_start(
                    out=wv_sb[:, :, :], in_=wvt.rearrange("(fc p) c -> p fc c", p=128)
                )
                prev_ctxn = None
                for qi in range(N_QC):
                    prev_ctxn = emit_attention(qi, prev_ctxn)
                for mq in range(4):
                    emit_phase3(4 * (N_QC - 1) + mq, tail=True)

    _split_multi_waits(nc)
    return nc


_CACHED = {}


def _build(reps=1):
    if reps not in _CACHED:
        nc = bass.Bass("TRN2", target_bir_lowering=False, debug=False)
        _CACHED[reps] = _emit_kernel(nc, reps)
    return _CACHED[reps]


def _perm_for_chunks():
    """c-dim permutation for the fp8 DoubleRow score layout.

    Chunk cc (128 W columns) covers head group hg=cc//2, row=cc%2:
    column p holds c = (4*hg + p//32)*64 + 32*row + (p%32)."""
    perm = np.empty(C, np.int64)
    for cc in range(NCC):
        hg, row = cc // 2, cc % 2
        p = np.arange(128)
        perm[cc * 128 : (cc + 1) * 128] = (4 * hg + p // 32) * 64 + 32 * row + (p % 32)
    return perm


_PERM = _perm_for_chunks()


def _reference_numpy(x, Wq, bq, Wk, bk, Wv, bv, Wo, bo, attention_mask):
    """Fallback for non-all-ones attention masks (spec fills ones)."""
    scale = HD ** -0.5
    out = np.empty((B, S, D), np.float32)
    causal = np.triu(np.ones((S, S), bool), k=1)
    for b in range(B):
        q = (x[b] @ Wq.T + bq).reshape(S, H, HD).transpose(1, 0, 2)
        k = (x[b] @ Wk.T + bk).reshape(S, H, HD).transpose(1, 0, 2)
        v = (x[b] @ Wv.T + bv).reshape(S, H, HD).transpose(1, 0, 2)
        o = np.empty((H, S, HD), np.float32)
        pad = (attention_mask[b] == 0)[None, :]
        for h in range(H):
            s = (q[h] @ k[h].T) * scale
            s[causal] = -np.inf
            s = np.where(pad, np.float32(-1e9), s)
            s -= s.max(-1, keepdims=True)
            e = np.exp(s)
            p = e / e.sum(-1, keepdims=True)
            o[h] = p @ v[h]
        ctx = o.transpose(1, 0, 2).reshape(S, D)
        out[b] = ctx @ Wo.T + bo
    return out


def kernel(x, Wq, bq, Wk, bk, Wv, bv, Wo, bo, attention_mask):
    x = np.asarray(x, np.float32)
    Wq, bq = np.asarray(Wq, np.float32), np.asarray(bq, np.float32)
    Wk, bk = np.asarray(Wk, np.float32), np.asarray(bk, np.float32)
    Wv, bv = np.asarray(Wv, np.float32), np.asarray(bv, np.float32)
    Wo, bo = np.asarray(Wo, np.float32), np.asarray(bo, np.float32)
    attention_mask = np.asarray(attention_mask)

    if not np.all(attention_mask == 1):
        return _reference_numpy(x, Wq, bq, Wk, bk, Wv, bv, Wo, bo, attention_mask)

    nc = _build()

    E4NP = ml_dtypes.float8_e4m3
    BFNP = ml_dtypes.bfloat16
    xbts = [np.ascontiguousarray(x[b].T.astype(BFNP)) for b in range(B)]
    x8ts = [np.ascontiguousarray(x[b].T.astype(E4NP)) for b in range(B)]
    shards = []
    for g in range(2):
        cs = slice(g * C, (g + 1) * C)
        Wq_c, Wk_c = Wq[cs, :][_PERM], Wk[cs, :][_PERM]
        bq_c, bk_c = bq[cs][_PERM], bk[cs][_PERM]
        shards.append(
            dict(
                wq8=np.ascontiguousarray((Wq_c * W_SCALE).T).astype(E4NP),
                wk8=np.ascontiguousarray((Wk_c * W_SCALE).T).astype(E4NP),
                wvt=np.ascontiguousarray(Wv[cs, :].T.astype(BFNP)),
                bqv=np.ascontiguousarray(
                    (bq_c * QK_STORE).reshape(NCC, 128).T
                ),
                bkv=np.ascontiguousarray(
                    (bk_c * QK_STORE).reshape(NCC, 128).T
                ),
                bvb=np.ascontiguousarray(np.broadcast_to(bv[cs], (128, C))),
                wot=np.ascontiguousarray(Wo[:, cs].T).astype(BFNP),
            )
        )
    in_maps = []
    for c in range(N_CORES):
        b, g = c // 2, c % 2
        in_maps.append(dict(xbt=xbts[b], x8t=x8ts[b], **shards[g]))

    res = run_bass_kernel_spmd(nc, in_maps, core_ids=list(range(N_CORES)))

    out = np.empty((B, S, D), np.float32)
    for b in range(B):
        out[b] = res.results[2 * b]["out"] + res.results[2 * b + 1]["out"] + bo
    return out
